# revision 1
# baseline (speedup 1.0000x reference)
"""DeformConv2d (B=8, C=128, H=W=64, K=3x3, pad 1, stride 1) on 8 trn2 NeuronCores.

Data-parallel over batch: core b handles image b. Per core:
  - x image zero-padded with a 2-pixel ring (rows/cols -2..65) so the
    reference's out-of-bounds corner masking is exactly reproduced by the
    padding (clamped sample indices land in the zero ring).
  - P2[i] packs (bf16(XP[i]), bf16(XP[i+68])) into one fp32 word, so a single
    gpsimd ap_gather index fetches the vertical corner pair (y0,x)/(y0+1,x).
    A second gather at lin+1 fetches the (x0+1) pair: all 4 bilinear corners
    in 2 gathers per kernel tap.
  - Bilinear weights (pure fraction products; no masks needed) are computed
    on DVE in a [128, 288] layout (position%128 on partitions), staged to
    DRAM, and broadcast to all 128 partitions with a stride-0-source DMA.
  - Per (quarter, tap): weighted corner products in bf16 (DVE 2x mode), then
    PE matmuls accumulate both corner-pair lanes and all 9 taps into PSUM.
  - Tail per quarter: even+odd PSUM lanes + bias -> fp32 output.
"""
import numpy as np
import ml_dtypes

B, CIN, H, W = 8, 128, 64, 64
COUT, KH, KW = 128, 3, 3
K = KH * KW
HO, WO = 64, 64
P = 128                      # partitions
NPOS = HO * WO               # 4096 output positions per image
Q = NPOS // P                # 32 free-dim columns in the [128, 288] gen layout
PADR = 2                     # zero-pad ring width
HP = H + 2 * PADR            # 68
WP = W + 2 * PADR            # 68
NE = HP * WP                 # 4624 padded elements
NXP = NE + WP + 1            # XP alloc with tail zeros for corner shifts
NQT = NPOS // 4              # 1024 positions per PSUM quarter
QI = NQT // 16               # 64 idx-cols per quarter
FB = 1024.0                  # floor-trick bias constant


def _build_kernel(repeat=1):
    import concourse.bacc as bacc
    import concourse.mybir as mybir
    import concourse.tile as tile
    import concourse.library_config as library_config

    nc = bacc.Bacc("TRN2", target_bir_lowering=False, debug=False, num_devices=8)
    f32, bf16, i16 = mybir.dt.float32, mybir.dt.bfloat16, mybir.dt.int16
    ALU = mybir.AluOpType

    x_d = nc.dram_tensor("x", [P, NPOS], f32, kind="ExternalInput")
    off_d = nc.dram_tensor("offs", [2 * K, NPOS], f32, kind="ExternalInput")
    wmat_d = nc.dram_tensor("wmat", [P, K * COUT], bf16, kind="ExternalInput")
    bias_d = nc.dram_tensor("bias", [P, 1], f32, kind="ExternalInput")
    hob_d = nc.dram_tensor("hob", [P, K * Q], f32, kind="ExternalInput")
    wob_d = nc.dram_tensor("wob", [P, K * Q], f32, kind="ExternalInput")
    out_d = nc.dram_tensor("out", [P, NPOS], f32, kind="ExternalOutput")

    with tile.TileContext(nc) as tc:
        with tc.tile_pool(name="const", bufs=1) as cpool, \
             tc.tile_pool(name="gen", bufs=1) as gpool, \
             tc.tile_pool(name="wbc", bufs=3) as wpool, \
             tc.tile_pool(name="gath", bufs=3) as gapool, \
             tc.tile_pool(name="mm", bufs=3) as mpool, \
             tc.tile_pool(name="outp", bufs=2) as opool, \
             tc.tile_pool(name="dramw", bufs=1, space="DRAM") as dpool, \
             tc.tile_pool(name="ps", bufs=1, space="PSUM") as pspool:

            # staging for weight rows: [k, 4*NPOS] 4-lane interleaved, natural p
            wrow = dpool.tile([K, 4 * NPOS], mybir.dt.bfloat16)

            nc.gpsimd.load_library(library_config.ap_gather)

            for _rep in range(repeat):
              # -------------- stage 0: loads + padded image + P2 pack -------
              XP = cpool.tile([P, NXP], f32)
              nc.vector.memset(XP[:], 0.0)
              # x rows into the padded interior (strided dst AP, one DMA)
              xp_img = XP[:, 0:NE].rearrange("p (h w) -> p h w", h=HP, w=WP)
              nc.sync.dma_start(
                  out=xp_img[:, PADR : PADR + H, PADR : PADR + W],
                  in_=x_d.ap().rearrange("p (h w) -> p h w", h=H, w=W),
              )

              wmat = cpool.tile([P, K * COUT], bf16)
              nc.sync.dma_start(out=wmat[:], in_=wmat_d.ap())
              bias = cpool.tile([P, 1], f32)
              nc.sync.dma_start(out=bias[:], in_=bias_d.ap())
              hob = cpool.tile([P, K * Q], f32)
              nc.sync.dma_start(out=hob[:], in_=hob_d.ap())
              wob = cpool.tile([P, K * Q], f32)
              nc.sync.dma_start(out=wob[:], in_=wob_d.ap())
              # permuted offsets: offy[Pp, k*Q+q] = off[2k, q*128+Pp], offx likewise
              offy = cpool.tile([P, K * Q], f32)
              offx = cpool.tile([P, K * Q], f32)
              for k in range(K):
                  nc.scalar.dma_start(
                      out=offy[:, k * Q : (k + 1) * Q],
                      in_=off_d.ap()[2 * k].rearrange("(q p) -> p q", p=P),
                  )
                  nc.scalar.dma_start(
                      out=offx[:, k * Q : (k + 1) * Q],
                      in_=off_d.ap()[2 * k + 1].rearrange("(q p) -> p q", p=P),
                  )

              # P4 pack: 4 bf16 corner lanes per index in two fp32 words:
              # lanes (XP[i], XP[i+WP], XP[i+1], XP[i+WP+1]) = (A, C, B, D)
              P4 = cpool.tile([P, 2 * NE], f32)
              p4h = P4[:].bitcast(mybir.dt.bfloat16)  # [P, 4*NE]
              nc.scalar.copy(out=p4h[:, 0 : 4 * NE : 4], in_=XP[:, 0:NE])
              nc.scalar.copy(out=p4h[:, 1 : 4 * NE : 4], in_=XP[:, WP : NE + WP])
              nc.scalar.copy(out=p4h[:, 2 : 4 * NE : 4], in_=XP[:, 1 : NE + 1])
              nc.scalar.copy(out=p4h[:, 3 : 4 * NE : 4],
                             in_=XP[:, WP + 1 : NE + WP + 1])

              # ---------------- stage 1: weights + indices -------------------
              NG = K * Q  # 288
              pyb = gpool.tile([P, NG], f32)
              pxb = gpool.tile([P, NG], f32)
              # pyb = (offy + FB) + hob   (hob already holds ho - 1 + ky)
              nc.vector.scalar_tensor_tensor(
                  out=pyb[:], in0=offy[:], scalar=FB, in1=hob[:],
                  op0=ALU.add, op1=ALU.add)
              nc.vector.scalar_tensor_tensor(
                  out=pxb[:], in0=offx[:], scalar=FB, in1=wob[:],
                  op0=ALU.add, op1=ALU.add)
              # floor robust to cast rounding mode (trunc in sim, RN on hw):
              # y0 = cast(pyb); lyr = pyb - y0; adj = (lyr < 0); floor = y0 - adj
              def floor_frac(pb, sfx):
                  i0 = gpool.tile([P, NG], mybir.dt.int32, tag="ffi" + sfx)
                  nc.vector.tensor_copy(out=i0[:], in_=pb[:])
                  f0 = gpool.tile([P, NG], f32, tag="fff" + sfx)
                  nc.vector.tensor_copy(out=f0[:], in_=i0[:])
                  lr = gpool.tile([P, NG], f32, tag="ffl" + sfx)
                  nc.vector.tensor_tensor(out=lr[:], in0=pb[:], in1=f0[:],
                                          op=ALU.subtract)
                  adj = gpool.tile([P, NG], f32, tag="ffa" + sfx)
                  nc.vector.tensor_scalar(out=adj[:], in0=lr[:], scalar1=0.0,
                                          scalar2=None, op0=ALU.is_lt)
                  fr = gpool.tile([P, NG], f32, tag="ffr" + sfx)
                  nc.vector.tensor_tensor(out=fr[:], in0=lr[:], in1=adj[:],
                                          op=ALU.add)
                  fl = gpool.tile([P, NG], f32, tag="ffo" + sfx)
                  nc.vector.tensor_tensor(out=fl[:], in0=f0[:], in1=adj[:],
                                          op=ALU.subtract)
                  return fl, fr
              y0f, ly = floor_frac(pyb, "y")
              x0f, lx = floor_frac(pxb, "x")
              omly = gpool.tile([P, NG], f32)
              omlx = gpool.tile([P, NG], f32)
              nc.vector.tensor_scalar(out=omly[:], in0=ly[:], scalar1=-1.0, scalar2=1.0,
                                      op0=ALU.mult, op1=ALU.add)
              nc.vector.tensor_scalar(out=omlx[:], in0=lx[:], scalar1=-1.0, scalar2=1.0,
                                      op0=ALU.mult, op1=ALU.add)
              # clamp biased corner coords to [-PADR, 64]+FB
              ycl = gpool.tile([P, NG], f32)
              xcl = gpool.tile([P, NG], f32)
              nc.vector.tensor_scalar(out=ycl[:], in0=y0f[:], scalar1=FB - PADR,
                                      scalar2=FB + 64.0, op0=ALU.max, op1=ALU.min)
              nc.vector.tensor_scalar(out=xcl[:], in0=x0f[:], scalar1=FB - PADR,
                                      scalar2=FB + 64.0, op0=ALU.max, op1=ALU.min)
              # lin = (ycl-FB+PADR)*WP + (xcl-FB+PADR) = WP*ycl + xcl - (WP+1)*(FB-PADR)
              linf = gpool.tile([P, NG], f32)
              nc.vector.scalar_tensor_tensor(
                  out=linf[:], in0=ycl[:], scalar=float(WP), in1=xcl[:],
                  op0=ALU.mult, op1=ALU.add)
              linf2 = gpool.tile([P, NG], f32)
              nc.vector.tensor_scalar(out=linf2[:], in0=linf[:],
                                      scalar1=-(WP + 1.0) * (FB - PADR),
                                      scalar2=None, op0=ALU.add)
              lin16 = gpool.tile([P, NG], i16)
              nc.vector.tensor_copy(out=lin16[:], in_=linf2[:])

              # weight products, 4-lane interleave matching P4 lane order
              wpre_cat = gpool.tile([P, 4 * NG], bf16)
              wv = wpre_cat[:].rearrange("p (k q j) -> p k q j",
                                         k=K, q=Q, j=4)
              omly3 = omly[:].rearrange("p (k q) -> p k q", k=K, q=Q)
              ly3 = ly[:].rearrange("p (k q) -> p k q", k=K, q=Q)
              omlx3 = omlx[:].rearrange("p (k q) -> p k q", k=K, q=Q)
              lx3 = lx[:].rearrange("p (k q) -> p k q", k=K, q=Q)
              nc.vector.tensor_tensor(out=wv[:, :, :, 0], in0=omly3, in1=omlx3,
                                      op=ALU.mult)  # w00 (A)
              nc.vector.tensor_tensor(out=wv[:, :, :, 1], in0=ly3, in1=omlx3,
                                      op=ALU.mult)  # w10 (C)
              nc.vector.tensor_tensor(out=wv[:, :, :, 2], in0=omly3, in1=lx3,
                                      op=ALU.mult)  # w01 (B)
              nc.vector.tensor_tensor(out=wv[:, :, :, 3], in0=ly3, in1=lx3,
                                      op=ALU.mult)  # w11 (D)
              # stage to DRAM: wrow[k, (q*128+Pp)*4 + j] = wpre_cat[Pp, kqj]
              wrow_v = wrow[:].rearrange(
                  "k (q p j) -> p k q j", k=K, q=Q, p=P, j=4)
              nc.sync.dma_start(out=wrow_v, in_=wpre_cat[:])

              # index tensors: wrapped-16 layout for ap_gather
              # idxw[16g + r, k*(8Q) + 8q + u] = lin16[16u + r, k*Q + q]
              NI = 8 * K * Q  # 2304 idx-cols total (256 per tap)
              idxw = gpool.tile([P, NI], i16)
              for u in range(8):
                  nc.scalar.dma_start(
                      out=idxw[0:16, :].rearrange(
                          "p (k q u) -> p k q u", k=K, q=Q, u=8)[:, :, :, u],
                      in_=lin16[16 * u : 16 * u + 16, :].rearrange(
                          "p (k q) -> p k q", k=K, q=Q),
                  )
              for g in range(1, 8):
                  nc.scalar.dma_start(out=idxw[16 * g : 16 * g + 16, :],
                                      in_=idxw[0:16, :])


              # ---------------- stage 2+3: gather/mul/matmul per quarter -----
              for qt in range(4):
                  psum = pspool.tile([P, 4 * NQT], f32, tag="ps")
                  for k in range(K):
                      # broadcast this (tap, quarter) 4-lane weight slice
                      wbc = wpool.tile([P, 4 * NQT], bf16, tag="wb")
                      sl = slice(qt * 4 * NQT, (qt + 1) * 4 * NQT)
                      nc.sync.dma_start(
                          out=wbc[:],
                          in_=wrow[k : k + 1, sl].to_broadcast((P, 4 * NQT)))
                      i0 = k * 4 * QI + qt * QI
                      g4 = gapool.tile([P, 2 * NQT], f32, tag="g")
                      nc.gpsimd.ap_gather(
                          g4[:], P4[:], idxw[:, i0 : i0 + QI],
                          channels=P, num_elems=NE, d=2, num_idxs=NQT)
                      m = mpool.tile([P, 4 * NQT], bf16, tag="m")
                      nc.vector.tensor_tensor(
                          out=m[:],
                          in0=g4[:].bitcast(mybir.dt.bfloat16),
                          in1=wbc[:], op=ALU.mult)
                      lhsT = wmat[:, k * COUT : (k + 1) * COUT]
                      for bk in range(8):  # 512-col pieces, one PSUM bank each
                          c0 = bk * 512
                          nc.tensor.matmul(
                              psum[:, c0 : c0 + 512], lhsT,
                              m[:, c0 : c0 + 512],
                              start=(k == 0), stop=(k == K - 1),
                              skip_group_check=True)
                  # tail: sum 4 corner lanes + bias -> fp32 out
                  pv = psum[:].rearrange("p (n j) -> p n j", j=4)
                  t = opool.tile([P, NQT], f32, tag="t")
                  nc.vector.tensor_scalar(
                      out=t[:], in0=pv[:, :, 0],
                      scalar1=bias[:, 0:1], scalar2=None, op0=ALU.add)
                  t2 = opool.tile([P, NQT], f32, tag="t2")
                  nc.vector.tensor_tensor(
                      out=t2[:], in0=t[:], in1=pv[:, :, 1], op=ALU.add)
                  t3 = opool.tile([P, NQT], f32, tag="t3")
                  nc.vector.tensor_tensor(
                      out=t3[:], in0=t2[:], in1=pv[:, :, 2], op=ALU.add)
                  o = opool.tile([P, NQT], f32, tag="o")
                  nc.vector.tensor_tensor(
                      out=o[:], in0=t3[:], in1=pv[:, :, 3], op=ALU.add)
                  nc.sync.dma_start(
                      out=out_d.ap()[:, qt * NQT : (qt + 1) * NQT], in_=o[:])

    nc.compile()
    return nc


_NC_CACHE = None


def _host_inputs(x, offset, weight, bias):
    """Per-core input maps (core b <- batch b) + replicated constants."""
    wq = np.ascontiguousarray(weight, np.float32)  # [COUT, CIN, KH, KW]
    # wmat[c, k*COUT + o] = weight[o, c, ky, kx]
    wmat = wq.reshape(COUT, CIN, K).transpose(1, 2, 0).reshape(CIN, K * COUT)
    wmat = np.ascontiguousarray(wmat).astype(ml_dtypes.bfloat16)
    bias_h = np.ascontiguousarray(bias, np.float32).reshape(P, 1)
    # hob[Pp, k*Q+q] = ho(p) - 1 + ky,  wob = wo(p) - 1 + kx,  p = q*128 + Pp
    p_of = (np.arange(Q)[:, None] * P + np.arange(P)[None, :])  # [Q, P]
    ho = (p_of // WO).astype(np.float32)
    wo = (p_of % WO).astype(np.float32)
    hob = np.empty((P, K * Q), np.float32)
    wob = np.empty((P, K * Q), np.float32)
    for k in range(K):
        hob[:, k * Q : (k + 1) * Q] = (ho + (k // 3 - 1)).T
        wob[:, k * Q : (k + 1) * Q] = (wo + (k % 3 - 1)).T
    in_maps = []
    for b in range(B):
        in_maps.append({
            "x": np.ascontiguousarray(x[b], np.float32).reshape(P, NPOS),
            "offs": np.ascontiguousarray(offset[b], np.float32).reshape(2 * K, NPOS),
            "wmat": wmat,
            "bias": bias_h,
            "hob": hob,
            "wob": wob,
        })
    return in_maps


def kernel(x, offset, weight, bias):
    global _NC_CACHE
    from concourse.bass_utils import run_bass_kernel_spmd

    if _NC_CACHE is None:
        _NC_CACHE = _build_kernel()
    nc = _NC_CACHE
    in_maps = _host_inputs(x, offset, weight, bias)
    res = run_bass_kernel_spmd(nc, in_maps, list(range(B)))
    out = np.stack([res.results[b]["out"].reshape(COUT, HO, WO) for b in range(B)])
    return out.astype(np.float32)


if __name__ == "__main__":
    import sys
    d = np.load("/tmp/inputs.npz")
    if len(sys.argv) > 1 and sys.argv[1] == "sim":
        from concourse.bass_interp import CoreSim
        nc = _build_kernel()
        in_maps = _host_inputs(d["x"], d["offset"], d["weight"], d["bias"])
        sim = CoreSim(nc)
        for kk, vv in in_maps[0].items():
            sim.tensor(kk)[:] = vv
        sim.simulate()
        out = np.asarray(sim.tensor("out")).reshape(1, COUT, HO, WO)
        exp = np.load("/tmp/expected.npy")[0:1]
    else:
        out = kernel(d["x"], d["offset"], d["weight"], d["bias"])
        exp = np.load("/tmp/expected.npy")
    err = np.abs(out - exp)
    print("rel l2:", np.linalg.norm(out - exp) / np.linalg.norm(exp))
    print("absmax rel:", err.max() / np.abs(exp).max())



# revision 4
# speedup vs baseline: 2.1502x; 2.1502x over previous
"""DeformConv2d (B=8, C=128, H=W=64, K=3x3, pad 1, stride 1) on 8 trn2 NeuronCores.

Data-parallel over batch: core b handles image b. Per core:
  - The padded image lives in DRAM position-major with all 4 bilinear corner
    rows packed per entry: xt4[r] = [pos r | r+1 | r+68 | r+69], each 128ch
    bf16, r = y*68+x over a 68x68 zero-ringed grid (ring width 2). One
    dma_gather index fetches all 4 corners of one sample point for all 128
    channels, transposed into channel-on-partition SBUF layout [c, l, i]
    (l = corner lane y0x0,y0x1,y1x0,y1x1).
  - Bilinear corner weights are computed on DVE in natural [128, 288] layout,
    staged to DRAM lane-major, and broadcast to all 128 partitions with a
    stride-0-source DMA (one 32KB broadcast per tap).
  - DVE multiplies gathered corners by broadcast weights (bf16); PE matmuls
    accumulate 9 taps x 4 corner lanes into per-quarter PSUM [128, 1024].
  - Tail: psum + bias -> fp32 output quarter, DMA to DRAM.
dma_gather note: transpose-mode gathers hang above ~1024 descriptors in
flight (SWDGE ring capacity); chunks are capped at 896 indices per call.
"""
import numpy as np
import ml_dtypes

B, CIN, H, W = 8, 128, 64, 64
COUT, KH, KW = 128, 3, 3
K = KH * KW
HO, WO = 64, 64
P = 128
NPOS = HO * WO              # 4096 positions
Q = NPOS // P               # 32 idx-cols in natural [128, K*Q] layout
PADR = 2
WP = W + 2 * PADR           # 68
HP = H + 2 * PADR           # 68
NE = HP * WP                # 4624 padded positions
FB = 1024.0                 # bias to keep pre-floor coords positive
NH = NPOS // 2              # 2048 positions per half
NQ = NPOS // 4              # 1024 positions per psum quarter
IDXC = K * Q * 8            # 2304 wrapped idx cols (k, q, g)
CHUNKS = [(0, 896), (896, 896), (1792, 256)]  # per-half gather chunks


def _build_kernel():
    import concourse.bacc as bacc
    import concourse.mybir as mybir
    import concourse.tile as tile
    import concourse.library_config as library_config

    nc = bacc.Bacc("TRN2", target_bir_lowering=False, debug=False, num_devices=8)
    f32, bf16, i16 = mybir.dt.float32, mybir.dt.bfloat16, mybir.dt.int16
    i32 = mybir.dt.int32
    ALU = mybir.AluOpType

    xt_d = nc.dram_tensor("xt", [NE, 4 * P], bf16, kind="ExternalInput")
    off_d = nc.dram_tensor("off2", [P, 2 * K * Q], f32, kind="ExternalInput")
    tab_d = nc.dram_tensor("tab2", [P, 2 * K * Q], f32, kind="ExternalInput")
    wmat_d = nc.dram_tensor("wmat", [P, K * COUT], bf16, kind="ExternalInput")
    bias_d = nc.dram_tensor("bias", [P, 1], f32, kind="ExternalInput")
    out_d = nc.dram_tensor("out", [P, NPOS], f32, kind="ExternalOutput")

    with tile.TileContext(nc) as tc:
        with tc.tile_pool(name="const", bufs=1) as cpool, \
             tc.tile_pool(name="gen", bufs=1) as gpool, \
             tc.tile_pool(name="wbc", bufs=2) as wpool, \
             tc.tile_pool(name="gath", bufs=6) as gapool, \
             tc.tile_pool(name="mm", bufs=2) as mpool, \
             tc.tile_pool(name="outp", bufs=2) as opool, \
             tc.tile_pool(name="dramw", bufs=1, space="DRAM") as dpool, \
             tc.tile_pool(name="ps", bufs=1, space="PSUM") as pspool:

            wrow = dpool.tile([K, 4 * NPOS], mybir.dt.bfloat16)

            nc.gpsimd.load_library(library_config.mlp)

            # ---------------- stage 0: input loads --------------------------
            off2 = cpool.tile([P, 2 * K * Q], f32)
            nc.sync.dma_start(out=off2[:], in_=off_d.ap())
            tab2 = cpool.tile([P, 2 * K * Q], f32)
            nc.sync.dma_start(out=tab2[:], in_=tab_d.ap())
            wmat = cpool.tile([P, K * COUT], bf16)
            nc.scalar.dma_start(out=wmat[:], in_=wmat_d.ap())
            bias = cpool.tile([P, 1], f32)
            nc.scalar.dma_start(out=bias[:], in_=bias_d.ap())

            # ---------------- stage 1: coords, weights, indices -------------
            NG = K * Q  # 288
            py = gpool.tile([P, NG], f32)
            px = gpool.tile([P, NG], f32)
            # py = offy + (hob + FB)   (tab already holds ho - 1 + ky + FB)
            nc.vector.tensor_tensor(out=py[:], in0=off2[:, 0:NG],
                                    in1=tab2[:, 0:NG], op=ALU.add)
            nc.vector.tensor_tensor(out=px[:], in0=off2[:, NG:2 * NG],
                                    in1=tab2[:, NG:2 * NG], op=ALU.add)

            # floor robust to cast rounding mode (trunc in sim, RN on hw)
            def floor_frac(pb, sfx):
                i0 = gpool.tile([P, NG], i32, tag="ffi" + sfx)
                nc.vector.tensor_copy(out=i0[:], in_=pb[:])
                f0 = gpool.tile([P, NG], f32, tag="fff" + sfx)
                nc.vector.tensor_copy(out=f0[:], in_=i0[:])
                lr = gpool.tile([P, NG], f32, tag="ffl" + sfx)
                nc.vector.tensor_tensor(out=lr[:], in0=pb[:], in1=f0[:],
                                        op=ALU.subtract)
                adj = gpool.tile([P, NG], f32, tag="ffa" + sfx)
                nc.vector.tensor_scalar(out=adj[:], in0=lr[:], scalar1=0.0,
                                        scalar2=None, op0=ALU.is_lt)
                fr = gpool.tile([P, NG], f32, tag="ffr" + sfx)
                nc.vector.tensor_tensor(out=fr[:], in0=lr[:], in1=adj[:],
                                        op=ALU.add)
                fl = gpool.tile([P, NG], f32, tag="ffo" + sfx)
                nc.vector.tensor_tensor(out=fl[:], in0=f0[:], in1=adj[:],
                                        op=ALU.subtract)
                return fl, fr
            y0f, ly = floor_frac(py, "y")
            x0f, lx = floor_frac(px, "x")

            # clamp biased corner coords to [-PADR, 64]+FB
            ycl = gpool.tile([P, NG], f32)
            xcl = gpool.tile([P, NG], f32)
            nc.vector.tensor_scalar(out=ycl[:], in0=y0f[:], scalar1=FB - PADR,
                                    scalar2=FB + 64.0, op0=ALU.max, op1=ALU.min)
            nc.vector.tensor_scalar(out=xcl[:], in0=x0f[:], scalar1=FB - PADR,
                                    scalar2=FB + 64.0, op0=ALU.max, op1=ALU.min)
            # row idx r = (ycl-FB+2)*68 + (xcl-FB+2) = 68*ycl + xcl - 69*FB + 138
            rf = gpool.tile([P, NG], f32)
            nc.vector.scalar_tensor_tensor(
                out=rf[:], in0=ycl[:], scalar=float(WP), in1=xcl[:],
                op0=ALU.mult, op1=ALU.add)
            r16 = gpool.tile([P, NG], i16)
            nc.vector.tensor_scalar(out=r16[:], in0=rf[:],
                                    scalar1=-(WP + 1.0) * FB + 2 * WP + 2.0,
                                    scalar2=None, op0=ALU.add)

            # bilinear corner-weight products, lane-major (l = 2*jy + jx)
            omy = gpool.tile([P, NG], f32)
            omx = gpool.tile([P, NG], f32)
            nc.vector.tensor_scalar(out=omy[:], in0=ly[:], scalar1=-1.0,
                                    scalar2=1.0, op0=ALU.mult, op1=ALU.add)
            nc.vector.tensor_scalar(out=omx[:], in0=lx[:], scalar1=-1.0,
                                    scalar2=1.0, op0=ALU.mult, op1=ALU.add)
            wpre = gpool.tile([P, K * 4 * Q], bf16)
            wv = wpre[:].rearrange("p (k l q) -> p k l q", k=K, l=4, q=Q)
            omy3 = omy[:].rearrange("p (k q) -> p k q", k=K, q=Q)
            ly3 = ly[:].rearrange("p (k q) -> p k q", k=K, q=Q)
            omx3 = omx[:].rearrange("p (k q) -> p k q", k=K, q=Q)
            lx3 = lx[:].rearrange("p (k q) -> p k q", k=K, q=Q)
            nc.vector.tensor_tensor(out=wv[:, :, 0], in0=omy3, in1=omx3,
                                    op=ALU.mult)  # y0 x0
            nc.vector.tensor_tensor(out=wv[:, :, 1], in0=omy3, in1=lx3,
                                    op=ALU.mult)  # y0 x1
            nc.vector.tensor_tensor(out=wv[:, :, 2], in0=ly3, in1=omx3,
                                    op=ALU.mult)  # y1 x0
            nc.vector.tensor_tensor(out=wv[:, :, 3], in0=ly3, in1=lx3,
                                    op=ALU.mult)  # y1 x1
            # stage to DRAM: wrow[k, l*4096 + q*128 + Pp] = wpre[Pp, k, l, q]
            wrow_v = wrow[:].rearrange(
                "k (l q p) -> p k l q", l=4, q=Q, p=P)
            nc.sync.dma_start(out=wrow_v, in_=wpre[:])

            # wrapped idx table: idx16[r, k*256 + q*8 + g] = r16[g*16+r, k*Q+q]
            # (gather slot i = q*128 + g*16 + r = position p, identity order)
            idx16 = gpool.tile([P, IDXC], i16)
            idxv = idx16[0:16, :].rearrange("p (k q g) -> p k q g",
                                            k=K, q=Q, g=8)
            r16v = r16[:].rearrange("(g p) (k q) -> g p k q", g=8, p=16,
                                    k=K, q=Q)
            dma_engs = [nc.sync, nc.scalar, nc.sync, nc.scalar,
                        nc.sync, nc.scalar, nc.sync, nc.scalar]
            for g in range(8):
                dma_engs[g].dma_start(out=idxv[:, :, :, g], in_=r16v[g])
            # replicate to all 128 partitions (16 -> 32 -> 64 -> 128)
            nc.sync.dma_start(out=idx16[16:32, :], in_=idx16[0:16, :])
            nc.scalar.dma_start(out=idx16[32:64, :], in_=idx16[0:32, :])
            nc.sync.dma_start(out=idx16[64:128, :], in_=idx16[0:64, :])

            # ---------------- stage 2: per-tap gather/mult/matmul -----------
            # psum quarters: (h, q) -> positions [h*2048 + q*1024, +1024)
            ps = [[None, None], [None, None]]
            for h in range(2):
                for q in range(2):
                    psq = pspool.tile([P, NQ], f32, tag=f"ps{h}{q}",
                                      name=f"ps{h}{q}")
                    ps[h][q] = psq

            for k in range(K):
                wbc = wpool.tile([P, 4 * NPOS], bf16, tag="wb")
                nc.sync.dma_start(
                    out=wbc[:],
                    in_=wrow[k:k + 1, :].to_broadcast((P, 4 * NPOS)))
                wbc4 = wbc[:].rearrange("p (l i) -> p l i", l=4)
                lhsT = wmat[:, k * COUT:(k + 1) * COUT]
                for h in range(2):
                    m = mpool.tile([P, 4 * NH], bf16, tag="m")
                    m3 = m[:].rearrange("p (l i) -> p l i", l=4)
                    for o, n in CHUNKS:
                        g = gapool.tile([P, 4 * 896], bf16, tag="g")
                        c0 = k * 256 + h * 128 + o // 16
                        nc.gpsimd.dma_gather(
                            g[:, :4 * n].rearrange("p (j i) -> p j i", j=4),
                            xt_d.ap(),
                            idx16[:, c0:c0 + n // 16],
                            num_idxs=n, num_idxs_reg=n, elem_size=4 * P,
                            transpose=True)
                        nc.vector.tensor_tensor(
                            out=m3[:, :, o:o + n],
                            in0=g[:, :4 * n].rearrange(
                                "p (l i) -> p l i", l=4),
                            in1=wbc4[:, :, h * NH + o:h * NH + o + n],
                            op=ALU.mult)
                    for q in range(2):
                        for l in range(4):
                            for b2 in range(2):
                                c0 = b2 * 512
                                nc.tensor.matmul(
                                    ps[h][q][:, c0:c0 + 512], lhsT,
                                    m[:, l * NH + q * NQ + c0:
                                      l * NH + q * NQ + c0 + 512],
                                    start=(k == 0 and l == 0),
                                    stop=(k == K - 1 and l == 3),
                                    skip_group_check=True)

            # ---------------- stage 3: bias + store -------------------------
            for h in range(2):
                for q in range(2):
                    o = opool.tile([P, NQ], f32, tag="o")
                    nc.vector.tensor_scalar(
                        out=o[:], in0=ps[h][q][:],
                        scalar1=bias[:, 0:1], scalar2=None, op0=ALU.add)
                    p0 = h * NH + q * NQ
                    nc.sync.dma_start(out=out_d.ap()[:, p0:p0 + NQ], in_=o[:])

    nc.compile()
    return nc


_NC_CACHE = None


def _host_inputs(x, offset, weight, bias):
    """Per-core input maps (core b <- batch b) + replicated constants."""
    wq = np.ascontiguousarray(weight, np.float32)  # [COUT, CIN, KH, KW]
    # wmat[c, k*COUT + o] = weight[o, c, ky, kx]
    wmat = wq.reshape(COUT, CIN, K).transpose(1, 2, 0).reshape(CIN, K * COUT)
    wmat = np.ascontiguousarray(wmat).astype(ml_dtypes.bfloat16)
    bias_h = np.ascontiguousarray(bias, np.float32).reshape(P, 1)
    # tab2[Pp, axis*288 + k*Q + q]: y: ho - 1 + ky + FB; x: wo - 1 + kx + FB
    p_of = (np.arange(Q)[:, None] * P + np.arange(P)[None, :])  # [Q, P]
    ho = (p_of // WO).astype(np.float32)
    wo = (p_of % WO).astype(np.float32)
    tab2 = np.empty((P, 2 * K * Q), np.float32)
    for k in range(K):
        tab2[:, k * Q:(k + 1) * Q] = (ho + (k // 3 - 1) + FB).T
        tab2[:, K * Q + k * Q:K * Q + (k + 1) * Q] = (wo + (k % 3 - 1) + FB).T
    in_maps = []
    for b in range(B):
        # off2[Pp, axis*288 + k*Q + q] = offset[b, 2k+axis, pos q*128+Pp]
        ob = np.ascontiguousarray(offset[b], np.float32).reshape(2 * K, NPOS)
        off2 = np.empty((P, 2 * K * Q), np.float32)
        oy = ob[0::2].reshape(K, Q, P)  # [k, q, Pp]
        ox = ob[1::2].reshape(K, Q, P)
        off2[:, :K * Q] = oy.transpose(2, 0, 1).reshape(P, K * Q)
        off2[:, K * Q:] = ox.transpose(2, 0, 1).reshape(P, K * Q)
        # xt4: padded position-major bf16, 4 corner rows packed per entry
        xp = np.zeros((HP, WP, P), np.float32)
        xp[PADR:PADR + H, PADR:PADR + W, :] = (
            np.ascontiguousarray(x[b], np.float32).transpose(1, 2, 0))
        flat = np.zeros((NE + WP + 2, P), np.float32)
        flat[:NE] = xp.reshape(NE, P)
        xt = np.empty((NE, 4 * P), np.float32)
        xt[:, 0 * P:1 * P] = flat[0:NE]
        xt[:, 1 * P:2 * P] = flat[1:NE + 1]
        xt[:, 2 * P:3 * P] = flat[WP:NE + WP]
        xt[:, 3 * P:4 * P] = flat[WP + 1:NE + WP + 1]
        in_maps.append({
            "xt": xt.astype(ml_dtypes.bfloat16),
            "off2": off2,
            "tab2": tab2,
            "wmat": wmat,
            "bias": bias_h,
        })
    return in_maps


def kernel(x, offset, weight, bias):
    global _NC_CACHE
    from concourse.bass_utils import run_bass_kernel_spmd

    if _NC_CACHE is None:
        _NC_CACHE = _build_kernel()
    nc = _NC_CACHE
    in_maps = _host_inputs(x, offset, weight, bias)
    res = run_bass_kernel_spmd(nc, in_maps, list(range(B)))
    out = np.stack([res.results[b]["out"].reshape(COUT, HO, WO) for b in range(B)])
    return out.astype(np.float32)


if __name__ == "__main__":
    import sys
    d = np.load("/tmp/inputs.npz")
    if len(sys.argv) > 1 and sys.argv[1] == "sim":
        from concourse.bass_interp import CoreSim
        nc = _build_kernel()
        in_maps = _host_inputs(d["x"], d["offset"], d["weight"], d["bias"])
        sim = CoreSim(nc)
        for kk, vv in in_maps[0].items():
            sim.tensor(kk)[:] = vv
        sim.simulate()
        out = np.asarray(sim.tensor("out")).reshape(1, COUT, HO, WO)
        exp = np.load("/tmp/expected.npy")[0:1]
    else:
        out = kernel(d["x"], d["offset"], d["weight"], d["bias"])
        exp = np.load("/tmp/expected.npy")
    err = np.abs(out - exp)
    print("rel l2:", np.linalg.norm(out - exp) / np.linalg.norm(exp))
    print("absmax rel:", err.max() / np.abs(exp).max())
